# revision 21
# baseline (speedup 1.0000x reference)
"""Trainium2 Bass kernel for nn_Attention_77824807403911 (sparse_attention).

Math (per batch element, no softmax => associativity):
    q = x @ Wq^T + bq ; v = x @ Wv^T + bv          [1024, 256]
    rq = rope(q) ; rv = rope(v)
    per head h (16 heads, hd=16):  att_h = rq_h @ (rq_h^T @ rv_h) / 4
    out = att @ Wo^T + bo

Instead of the 1024x1024 score matrix we compute the 16x16 Gram per head
realized as a full 256x256 Gram masked to the block-diagonal (mask is
pre-scaled by 1/4 on host), folded with Wo into one [256,256] weight:
    F[e,f]  = sum_s rv[s,e] rq[s,f]       (Gram, natural-layout operands)
    BDT     = F .* (blockmask/4)
    W2[f,o] = sum_e BDT[e,f] * Wo[o,e]
    outT    = W2^T @ rqT + bo             ([256, 1024])

Pipeline (2 seq halves of 512): chunked input DMA -> projections (PE) ->
rope (DVE/Pool, fused q|v ops on [128,2,512] APs) -> PE transposes into
PSUM-packed natural tiles -> Gram accumulation -> W2 -> final + store.
Output stored bf16 (upcast on host). Data-parallel over batch, 1 element
per core, no collectives.
"""

import numpy as np
import ml_dtypes

import concourse.bass as bass
import concourse.bacc as bacc
import concourse.tile as tile
from concourse import mybir
from concourse.bass_utils import run_bass_kernel_spmd
from concourse.masks import make_identity

B, S, D, H, HD = 8, 1024, 256, 16, 16
N_CORES = 8
BF16 = mybir.dt.bfloat16
F32 = mybir.dt.float32

# wbig column layout (per 128-partition chunk): [bias3 | wqt | wvt | wot | mask]
BIAS0, WQ0, WV0, WO0, MK0, WCOLS = 0, 3, 259, 515, 771, 1027

# channel permutation: [evens of pairs 0-63 (theta=1), evens of pairs 64-127
# (theta=1e-4), odds of pairs 0-63, odds of pairs 64-127]
PERM = np.concatenate(
    [np.arange(0, 128, 2), np.arange(128, 256, 2),
     np.arange(1, 128, 2), np.arange(129, 256, 2)]
)


def _host_tables():
    s = np.arange(S, dtype=np.float64) + 1.0
    W = S + 128
    trig4 = np.zeros((2, 2 * W), np.float64)
    trig4[0, 0:S] = np.sin(s)           # sinA (theta=1)
    trig4[1, 0:S] = np.sin(1e-4 * s)    # sinB
    trig4[0, W:W + S] = np.cos(s)
    trig4[1, W:W + S] = np.cos(1e-4 * s)
    p = np.arange(128)
    trig4[0, S:W] = (p < 64); trig4[1, S:W] = (p >= 64)
    trig4[0, W + S:] = (p < 64); trig4[1, W + S:] = (p >= 64)
    trig4 = trig4.astype(ml_dtypes.bfloat16)
    a = np.arange(256)
    headp = (a % 128) // 8
    mask = (0.25 * (headp[:, None] == headp[None, :])).astype(ml_dtypes.bfloat16)
    return trig4, mask


def build_kernel():
    nc = bacc.Bacc()
    xT = nc.declare_dram_parameter("xT", [D, S], BF16, isOutput=False)
    wbig = nc.declare_dram_parameter("wbig", [D, WCOLS], BF16, isOutput=False)
    # trig rows [A; B], cols: [sin(1024) | E(128) | cos(1024) | E(128)]
    trig = nc.declare_dram_parameter("trig", [2, 2 * (S + 128)], BF16, isOutput=False)
    outT = nc.declare_dram_parameter("outT", [D, S], BF16, isOutput=True)

    with tile.TileContext(nc) as tc:
        _body(tc, xT, wbig, trig, outT)
    nc.compile()
    return nc


def _body(tc, xT, wbig, trig, outT):
    nc = tc.nc
    HS = S // 2  # 512, one half

    with (
        tc.tile_pool(name="const", bufs=1) as cpool,
        tc.tile_pool(name="acts", bufs=1) as apool,
        tc.tile_pool(name="psum", bufs=1, space="PSUM") as pp,
        tc.tile_pool(name="outp", bufs=4) as opool,
    ):
        # ---- warm-up scratch first: warm matmuls gate only on this ----
        scratch = cpool.tile([128, 512], BF16, tag="scratch", name="scratch")
        nc.gpsimd.memset(scratch[:], 0.25)
        ident = cpool.tile([128, 128], BF16, tag="ident", name="ident")

        # PSUM tiles.  Budget (per partition): proj 2x2KB + tp 2x2KB +
        # gram 2x1KB + w2 2KB + fin 2x2KB = 16KB = all 8 banks.
        def proj_ps():
            return pp.tile([128, HS], F32, tag="proj", bufs=2, name="proj_ps")

        def tp_ps():
            return pp.tile([128, 1024], BF16, tag="tp", bufs=2, name="tp_ps")

        # "acc" tag rotates two banks through: trig broadcasts -> the two
        # gram accumulation groups -> w2
        gram_ps = None

        def fin_ps():
            return pp.tile([128, HS], F32, tag="fin", bufs=2, name="fin_ps")

        # PE warm-up: garbage matmuls release the HAM clock gate while the
        # inputs stream in. scratch is memset in the preamble, so these run
        # before any input lands.
        warm_ps = fin_ps()
        for wi in range(7):
            nc.tensor.matmul(warm_ps[:], scratch[:, 0:128], scratch[:],
                             start=True, stop=True, skip_group_check=True)

        def dep_filler(ap):
            # tiny matmul reading `ap`: executes when ap's producer fires,
            # trickling PE activity through waits to hold the HAM clock open
            wp = proj_ps()
            nc.tensor.matmul(wp[:, 0:128], ap, scratch[:, 0:128],
                             start=True, stop=True, skip_group_check=True)

        # ---- input tiles + chunked DMAs, ordered for earliest first use ----
        xT_sb = [cpool.tile([128, S], BF16, tag=f"xT{cc}", name=f"xT{cc}")
                 for cc in range(2)]
        w_sb = [cpool.tile([128, WCOLS], BF16, tag=f"w{cc}", name=f"w{cc}")
                for cc in range(2)]
        trig4 = cpool.tile([2, 2 * (S + 128)], BF16, tag="trig4", name="trig4")
        trig_sb = cpool.tile([128, 2 * S], BF16, tag="trig", name="trig_sb")
        sin_sb = trig_sb[:, 0:S]
        cos_sb = trig_sb[:, S:2 * S]

        # sync queue: wq+bias(p0), xT(p0,h0), xT(p0,h1a), wv(p0), wrest(p0)
        HQ = HS // 2
        nc.sync.dma_start(w_sb[0][:, 0:WV0], wbig[0:128, 0:WV0])
        nc.sync.dma_start(xT_sb[0][:, 0:HS], xT[0:128, 0:HS])
        nc.sync.dma_start(xT_sb[0][:, HS:HS + HQ], xT[0:128, HS:HS + HQ])
        nc.sync.dma_start(w_sb[0][:, WV0:WO0], wbig[0:128, WV0:WO0])
        nc.sync.dma_start(w_sb[0][:, WO0:WCOLS], wbig[0:128, WO0:WCOLS])
        # scalar queue: wq+bias(p1), xT(p1,h0), xT(p0,h1b), wv(p1)
        nc.scalar.dma_start(w_sb[1][:, 0:WV0], wbig[128:256, 0:WV0])
        nc.scalar.dma_start(xT_sb[1][:, 0:HS], xT[128:256, 0:HS])
        nc.scalar.dma_start(xT_sb[0][:, HS + HQ:S], xT[0:128, HS + HQ:S])
        nc.scalar.dma_start(w_sb[1][:, WV0:WO0], wbig[128:256, WV0:WO0])
        # gpsimd queue (SWDGE): trig4 (tiny, first), xT(p1,h1), wrest(p1)
        nc.gpsimd.dma_start(trig4[:], trig[:])
        nc.gpsimd.dma_start(xT_sb[1][:, HS:S], xT[128:256, HS:S])
        nc.gpsimd.dma_start(w_sb[1][:, WO0:WCOLS], wbig[128:256, WO0:WCOLS])
        make_identity(nc, ident[:])

        def acc_ps():
            return pp.tile([128, 2 * D], F32, tag="acc", bufs=2, name="acc_ps")

        # broadcast the 2-unique-row trig tables to [128, 1024] via tiny
        # matmuls (fills the PE while xT h1 streams in)
        def trig_bcast():
            for ti, dst in ((0, sin_sb), (1, cos_sb)):
                base = ti * (S + 128)
                for hh in range(2):
                    ps = acc_ps()
                    nc.tensor.matmul(
                        ps[:, 0:HS],
                        trig4[:, base + S:base + S + 128],
                        trig4[:, base + hh * HS:base + (hh + 1) * HS],
                        start=True, stop=True,
                    )
                    eng = nc.scalar if hh == 0 else nc.vector
                    if eng is nc.scalar:
                        nc.scalar.activation(
                            dst[:, hh * HS:(hh + 1) * HS], ps[:, 0:HS],
                            mybir.ActivationFunctionType.Copy)
                    else:
                        nc.vector.tensor_copy(
                            dst[:, hh * HS:(hh + 1) * HS], ps[:, 0:HS])

        def wslice(base, cc, col0, ncol):
            return w_sb[cc][:, base + col0: base + col0 + ncol]

        def bias_ap(idx, cc):
            return w_sb[cc][:, BIAS0 + idx: BIAS0 + idx + 1]

        # f32 copies of the biases (tensor_scalar wants f32 scalar operands)
        biasf = [cpool.tile([128, 3], F32, tag=f"biasf{cc}", name=f"biasf{cc}")
                 for cc in range(2)]
        for cc in range(2):
            nc.gpsimd.tensor_copy(biasf[cc][:], w_sb[cc][:, 0:3])

        def bias_apf(idx, cc):
            return biasf[cc][:, idx: idx + 1]

        # fused q|v activation tiles: cols [q(1024) | v(1024)], E/O chunks
        qvE = apool.tile([128, 2 * S], BF16, tag="qvE", name="qvE")
        qvO = apool.tile([128, 2 * S], BF16, tag="qvO", name="qvO")
        rqvE = apool.tile([128, 2 * S], BF16, tag="rqvE", name="rqvE")
        rqvO = apool.tile([128, 2 * S], BF16, tag="rqvO", name="rqvO")
        qv = [qvE, qvO]
        rqv = [rqvE, rqvO]
        # rope temporaries (per half, rotated)
        t_sin = [apool.tile([128, S], BF16, tag=f"tsin{i}", name=f"tsin{i}")
                 for i in range(2)]
        t_cosE = [apool.tile([128, S], BF16, tag=f"tcE{i}", name=f"tcE{i}")
                  for i in range(2)]
        t_cosO = [apool.tile([128, S], BF16, tag=f"tcO{i}", name=f"tcO{i}")
                  for i in range(2)]
        t_sinO = [apool.tile([128, S], BF16, tag=f"tsO{i}", name=f"tsO{i}")
                  for i in range(2)]
        # natural-layout roped activations: [st 0..7] x [128 s, 256 chan]
        rq_nat = apool.tile([128, 2 * S], BF16, tag="rq_nat", name="rq_nat")
        rv_nat = apool.tile([128, 2 * S], BF16, tag="rv_nat", name="rv_nat")
        # block-diag gram (bf16) and folded weight
        bdt = [apool.tile([128, D], BF16, tag=f"bdt{ec}", name=f"bdt{ec}")
               for ec in range(2)]
        w2sb = apool.tile([128, 2 * D], BF16, tag="w2sb", name="w2sb")

        # ---- projections: qv[ac][:, p*1024 + s] = sum_d w[d, a] x[d, s] ----
        # one tensor (q or v), one half; evicts split across ACT and DVE so
        # rope (gated on the last q evict) starts as early as possible
        def project(p_idx, h, dve_evict):
            wbase = (WQ0, WV0)[p_idx]
            for ac in range(2):
                ps = proj_ps()
                for dc in range(2):
                    nc.tensor.matmul(
                        ps[:],
                        wslice(wbase, dc, ac * 128, 128),
                        xT_sb[dc][:, h * HS:(h + 1) * HS],
                        start=(dc == 0), stop=(dc == 1),
                    )
                dst = qv[ac][:, p_idx * S + h * HS: p_idx * S + (h + 1) * HS]
                if dve_evict and ac == 1:
                    nc.vector.tensor_scalar_add(dst, ps[:], bias_apf(p_idx, ac))
                else:
                    nc.scalar.activation(
                        dst, ps[:],
                        mybir.ActivationFunctionType.Identity,
                        bias=bias_ap(p_idx, ac),
                    )

        # ---- rope for tensor p_idx (0=q cols 0:S, 1=v cols S:2S) ----
        # rE = E*sin - O*cos ; rO = E*cos + O*sin.  Plain 2D [128,1024] ops
        # (3D/broadcast APs drop the DVE 2x perf mode on HW).  The O*cos
        # mult goes to Pool so DVE finishes the sub/add chain sooner.
        def rope(p_idx):
            sl = slice(p_idx * S, (p_idx + 1) * S)
            ts, tcE = t_sin[p_idx][:], t_cosE[p_idx][:]
            tcO, tsO = t_cosO[p_idx][:], t_sinO[p_idx][:]
            E = qvE[:, sl]; O = qvO[:, sl]
            rE = rqvE[:, sl]; rO = rqvO[:, sl]
            nc.vector.tensor_tensor(ts, E, sin_sb, mybir.AluOpType.mult)
            nc.vector.tensor_tensor(tcO, O, cos_sb, mybir.AluOpType.mult)
            nc.vector.tensor_tensor(rE, ts, tcO, mybir.AluOpType.subtract)
            nc.vector.tensor_tensor(tcE, E, cos_sb, mybir.AluOpType.mult)
            nc.vector.tensor_tensor(tsO, O, sin_sb, mybir.AluOpType.mult)
            nc.vector.tensor_tensor(rO, tcE, tsO, mybir.AluOpType.add)

        # ---- PE transposes into PSUM-packed natural tiles, E/O split ----
        def transp_cc(tp, h, p_idx, cc):
            for stl in range(4):
                st = h * 4 + stl
                nc.tensor.transpose(
                    tp[:, stl * D + cc * 128: stl * D + (cc + 1) * 128],
                    rqv[cc][:, p_idx * S + st * 128: p_idx * S + (st + 1) * 128],
                    ident[:],
                )

        def transp_evict(tp, h, dst_nat, evict_engine):
            if evict_engine is nc.scalar:
                nc.scalar.activation(
                    dst_nat[:, h * S:(h + 1) * S], tp[:],
                    mybir.ActivationFunctionType.Copy)
            else:
                evict_engine.tensor_copy(dst_nat[:, h * S:(h + 1) * S], tp[:])

        # ---- gram accumulation over the 4 s-tiles of half h ----
        def gram(h):
            nonlocal gram_ps
            if gram_ps is None:
                gram_ps = [acc_ps()[:, 0:D] for _ in range(2)]
            for stl in range(4):
                st = h * 4 + stl
                for ec in range(2):
                    nc.tensor.matmul(
                        gram_ps[ec],
                        rv_nat[:, st * D + ec * 128: st * D + (ec + 1) * 128],
                        rq_nat[:, st * D: (st + 1) * D],
                        start=(st == 0), stop=(st == 7),
                    )

        # ================== pipeline emission ==================
        project(0, 0, dve_evict=True)    # q h0
        trig_bcast()                     # fills PE while xT h1 streams in
        dep_filler(xT_sb[1][:, HS:HS + 128])
        dep_filler(xT_sb[0][:, HS:HS + 128])
        project(0, 1, dve_evict=True)    # q h1
        project(1, 0, dve_evict=False)   # v h0 (DVE starts rope q meanwhile)
        project(1, 1, dve_evict=False)   # v h1
        rope(0)   # q on DVE (~4.3us)
        rope(1)   # v on DVE (~4.3us more)
        # trickled fillers fire as rope ops complete, holding the clock; they
        # are ordered so no filler's dep is later than the work behind it
        dep_filler(t_sin[0][:, 0:128])
        dep_filler(t_cosE[0][:, 0:128])
        tpq = [tp_ps(), tp_ps()]
        transp_cc(tpq[0], 0, 0, 0)         # rq E tiles (after rqE ready)
        transp_cc(tpq[1], 1, 0, 0)
        dep_filler(t_sinO[0][:, 0:128])
        transp_cc(tpq[0], 0, 0, 1)         # rq O tiles
        transp_cc(tpq[1], 1, 0, 1)
        transp_evict(tpq[0], 0, rq_nat, nc.scalar)
        transp_evict(tpq[1], 1, rq_nat, nc.scalar)
        dep_filler(t_sin[1][:, 0:128])
        dep_filler(t_cosO[1][:, 0:128])
        dep_filler(t_cosE[1][:, 0:128])
        tpv = [tp_ps(), tp_ps()]
        transp_cc(tpv[0], 0, 1, 0)         # rv E tiles (after rvE ready)
        transp_cc(tpv[1], 1, 1, 0)
        dep_filler(t_sinO[1][:, 0:128])
        transp_cc(tpv[0], 0, 1, 1)         # rv O tiles
        transp_cc(tpv[1], 1, 1, 1)
        transp_evict(tpv[0], 0, rv_nat, nc.scalar)
        transp_evict(tpv[1], 1, rv_nat, nc.vector)
        dep_filler(rv_nat[:, 0:128])
        dep_filler(rv_nat[:, S:S + 128])
        gram(0)
        gram(1)

        # ---- BDT = gram .* (mask/4)  (Pool) ----
        for ec in range(2):
            nc.vector.tensor_tensor(
                bdt[ec][:], gram_ps[ec], wslice(MK0, ec, 0, D),
                mybir.AluOpType.mult)

        # ---- W2[f, o] = sum_e BDT[e, f] wot[e, o] ----
        w2_ps = acc_ps()
        for fc in range(2):
            for ec in range(2):
                nc.tensor.matmul(
                    w2_ps[:, fc * D:(fc + 1) * D],
                    bdt[ec][:, fc * 128:(fc + 1) * 128],
                    wslice(WO0, ec, 0, D),
                    start=(ec == 0), stop=(ec == 1),
                )
        nc.scalar.activation(w2sb[:, 0:D], w2_ps[:, 0:D],
                             mybir.ActivationFunctionType.Copy)
        nc.vector.tensor_copy(w2sb[:, D:2 * D], w2_ps[:, D:2 * D])
        dep_filler(w2sb[:, 0:128])

        # ---- final: outT[o, s] = sum_f W2[f, o] rqT[f, s] + bo ----
        for sc in range(2):
            for oc in range(2):
                ps = fin_ps()
                for fc in range(2):
                    nc.tensor.matmul(
                        ps[:],
                        w2sb[:, fc * D + oc * 128: fc * D + (oc + 1) * 128],
                        rqv[fc][:, sc * HS:(sc + 1) * HS],
                        start=(fc == 0), stop=(fc == 1),
                    )
                ot = opool.tile([128, HS], BF16, tag="out_sb", name="out_sb")
                if (sc + oc) % 2 == 0:
                    nc.scalar.activation(
                        ot[:], ps[:],
                        mybir.ActivationFunctionType.Identity,
                        bias=bias_ap(2, oc),
                    )
                else:
                    nc.vector.tensor_scalar_add(ot[:], ps[:], bias_apf(2, oc))
                outq = nc.sync if (2 * sc + oc) % 2 == 0 else nc.scalar
                outq.dma_start(
                    outT[oc * 128:(oc + 1) * 128, sc * HS:(sc + 1) * HS], ot[:])


_NC_CACHE = None


def _get_nc():
    global _NC_CACHE
    if _NC_CACHE is None:
        _NC_CACHE = build_kernel()
    return _NC_CACHE


def make_in_maps(x, wq_w, wq_b, wv_w, wv_b, wo_w, wo_b):
    trig4, mask = _host_tables()
    wq_p = np.ascontiguousarray(wq_w[PERM].T).astype(ml_dtypes.bfloat16)   # [d, a]
    wv_p = np.ascontiguousarray(wv_w[PERM].T).astype(ml_dtypes.bfloat16)
    wo_p = np.ascontiguousarray(wo_w[:, PERM].T).astype(ml_dtypes.bfloat16)  # [e, o]
    bias3 = np.stack([wq_b[PERM], wv_b[PERM], wo_b], axis=1).astype(ml_dtypes.bfloat16)
    wbig = np.ascontiguousarray(
        np.concatenate([bias3, wq_p, wv_p, wo_p, mask], axis=1))
    trig = np.ascontiguousarray(trig4)
    in_maps = []
    for b in range(B):
        in_maps.append({
            "xT": np.ascontiguousarray(x[b].T).astype(ml_dtypes.bfloat16),
            "wbig": wbig, "trig": trig,
        })
    return in_maps


TRACE = False
RUN_KWARGS = {}
LAST_RESULT = None


def kernel(x, wq_w, wq_b, wk_w, wk_b, wv_w, wv_b, wo_w, wo_b):
    global LAST_RESULT
    x = np.asarray(x, dtype=np.float32)
    in_maps = make_in_maps(x, np.asarray(wq_w, np.float32), np.asarray(wq_b, np.float32),
                           np.asarray(wv_w, np.float32), np.asarray(wv_b, np.float32),
                           np.asarray(wo_w, np.float32), np.asarray(wo_b, np.float32))
    nc = _get_nc()
    res = run_bass_kernel_spmd(nc, in_maps, core_ids=list(range(N_CORES)),
                               trace=TRACE, **RUN_KWARGS)
    LAST_RESULT = res
    outs = [np.ascontiguousarray(
        res.results[b]["outT"].astype(np.float32).T) for b in range(B)]
    return np.stack(outs).astype(np.float32)


# revision 22
# speedup vs baseline: 1.0231x; 1.0231x over previous
"""Trainium2 Bass kernel for nn_Attention_77824807403911 (sparse_attention).

Math (per batch element, no softmax => associativity):
    q = x @ Wq^T + bq ; v = x @ Wv^T + bv          [1024, 256]
    rq = rope(q) ; rv = rope(v)
    per head h (16 heads, hd=16):  att_h = rq_h @ (rq_h^T @ rv_h) / 4
    out = att @ Wo^T + bo

Instead of the 1024x1024 score matrix we compute the 16x16 Gram per head
realized as a full 256x256 Gram masked to the block-diagonal (mask is
pre-scaled by 1/4 on host), folded with Wo into one [256,256] weight:
    F[e,f]  = sum_s rv[s,e] rq[s,f]       (Gram, natural-layout operands)
    BDT     = F .* (blockmask/4)
    W2[f,o] = sum_e BDT[e,f] * Wo[o,e]
    outT    = W2^T @ rqT + bo             ([256, 1024])

Pipeline (2 seq halves of 512): chunked input DMA -> projections (PE) ->
rope (DVE/Pool, fused q|v ops on [128,2,512] APs) -> PE transposes into
PSUM-packed natural tiles -> Gram accumulation -> W2 -> final + store.
Output stored bf16 (upcast on host). Data-parallel over batch, 1 element
per core, no collectives.
"""

import numpy as np
import ml_dtypes

import concourse.bass as bass
import concourse.bacc as bacc
import concourse.tile as tile
from concourse import mybir
from concourse.bass_utils import run_bass_kernel_spmd
from concourse.masks import make_identity

B, S, D, H, HD = 8, 1024, 256, 16, 16
N_CORES = 8
BF16 = mybir.dt.bfloat16
F32 = mybir.dt.float32

# wbig column layout (per 128-partition chunk): [bias3 | wqt | wvt | wot | mask]
BIAS0, WQ0, WV0, WO0, MK0, WCOLS = 0, 3, 259, 515, 771, 1027

# channel permutation: [evens of pairs 0-63 (theta=1), evens of pairs 64-127
# (theta=1e-4), odds of pairs 0-63, odds of pairs 64-127]
PERM = np.concatenate(
    [np.arange(0, 128, 2), np.arange(128, 256, 2),
     np.arange(1, 128, 2), np.arange(129, 256, 2)]
)


def _host_tables():
    s = np.arange(S, dtype=np.float64) + 1.0
    W = S + 128
    trig4 = np.zeros((2, 2 * W), np.float64)
    trig4[0, 0:S] = np.sin(s)           # sinA (theta=1)
    trig4[1, 0:S] = np.sin(1e-4 * s)    # sinB
    trig4[0, W:W + S] = np.cos(s)
    trig4[1, W:W + S] = np.cos(1e-4 * s)
    p = np.arange(128)
    trig4[0, S:W] = (p < 64); trig4[1, S:W] = (p >= 64)
    trig4[0, W + S:] = (p < 64); trig4[1, W + S:] = (p >= 64)
    trig4 = trig4.astype(ml_dtypes.bfloat16)
    a = np.arange(256)
    headp = (a % 128) // 8
    mask = (0.25 * (headp[:, None] == headp[None, :])).astype(ml_dtypes.bfloat16)
    return trig4, mask


def build_kernel():
    nc = bacc.Bacc()
    xT = nc.declare_dram_parameter("xT", [D, S], BF16, isOutput=False)
    wbig = nc.declare_dram_parameter("wbig", [D, WCOLS], BF16, isOutput=False)
    # trig rows [A; B], cols: [sin(1024) | E(128) | cos(1024) | E(128)]
    trig = nc.declare_dram_parameter("trig", [2, 2 * (S + 128)], BF16, isOutput=False)
    outT = nc.declare_dram_parameter("outT", [D, S], BF16, isOutput=True)

    with tile.TileContext(nc) as tc:
        _body(tc, xT, wbig, trig, outT)
    nc.compile()
    return nc


def _body(tc, xT, wbig, trig, outT):
    nc = tc.nc
    HS = S // 2  # 512, one half

    with (
        tc.tile_pool(name="const", bufs=1) as cpool,
        tc.tile_pool(name="acts", bufs=1) as apool,
        tc.tile_pool(name="psum", bufs=1, space="PSUM") as pp,
        tc.tile_pool(name="outp", bufs=4) as opool,
    ):
        # ---- warm-up scratch first: warm matmuls gate only on this ----
        scratch = cpool.tile([128, 512], BF16, tag="scratch", name="scratch")
        nc.gpsimd.memset(scratch[:], 0.25)
        ident = cpool.tile([128, 128], BF16, tag="ident", name="ident")

        # PSUM tiles.  Budget (per partition): proj 2x2KB + tp 2x2KB +
        # gram 2x1KB + w2 2KB + fin 2x2KB = 16KB = all 8 banks.
        def proj_ps():
            return pp.tile([128, HS], F32, tag="proj", bufs=2, name="proj_ps")

        def tp_ps():
            return pp.tile([128, 1024], BF16, tag="tp", bufs=2, name="tp_ps")

        # "acc" tag rotates two banks through: trig broadcasts -> the two
        # gram accumulation groups -> w2
        gram_ps = None

        def fin_ps():
            return pp.tile([128, HS], F32, tag="fin", bufs=2, name="fin_ps")

        # PE warm-up: garbage matmuls release the HAM clock gate while the
        # inputs stream in. scratch is memset in the preamble, so these run
        # before any input lands.
        warm_ps = fin_ps()
        for wi in range(7):
            nc.tensor.matmul(warm_ps[:], scratch[:, 0:128], scratch[:],
                             start=True, stop=True, skip_group_check=True)

        def dep_filler(ap):
            # tiny matmul reading `ap`: executes when ap's producer fires,
            # trickling PE activity through waits to hold the HAM clock open
            wp = proj_ps()
            nc.tensor.matmul(wp[:, 0:128], ap, scratch[:, 0:128],
                             start=True, stop=True, skip_group_check=True)

        # ---- input tiles + chunked DMAs, ordered for earliest first use ----
        xT_sb = [cpool.tile([128, S], BF16, tag=f"xT{cc}", name=f"xT{cc}")
                 for cc in range(2)]
        w_sb = [cpool.tile([128, WCOLS], BF16, tag=f"w{cc}", name=f"w{cc}")
                for cc in range(2)]
        trig4 = cpool.tile([2, 2 * (S + 128)], BF16, tag="trig4", name="trig4")
        trig_sb = cpool.tile([128, 2 * S], BF16, tag="trig", name="trig_sb")
        sin_sb = trig_sb[:, 0:S]
        cos_sb = trig_sb[:, S:2 * S]

        # sync queue: wq+bias(p0), xT(p0,h0), xT(p0,h1a), wv(p0), wrest(p0)
        HQ = HS // 2
        nc.sync.dma_start(w_sb[0][:, 0:WV0], wbig[0:128, 0:WV0])
        nc.sync.dma_start(xT_sb[0][:, 0:HS], xT[0:128, 0:HS])
        nc.sync.dma_start(xT_sb[0][:, HS:HS + HQ], xT[0:128, HS:HS + HQ])
        nc.sync.dma_start(w_sb[0][:, WV0:WO0], wbig[0:128, WV0:WO0])
        nc.sync.dma_start(w_sb[0][:, WO0:WCOLS], wbig[0:128, WO0:WCOLS])
        # scalar queue: wq+bias(p1), xT(p1,h0), xT(p0,h1b), wv(p1)
        nc.scalar.dma_start(w_sb[1][:, 0:WV0], wbig[128:256, 0:WV0])
        nc.scalar.dma_start(xT_sb[1][:, 0:HS], xT[128:256, 0:HS])
        nc.scalar.dma_start(xT_sb[0][:, HS + HQ:S], xT[0:128, HS + HQ:S])
        nc.scalar.dma_start(w_sb[1][:, WV0:WO0], wbig[128:256, WV0:WO0])
        # gpsimd queue (SWDGE): trig4 (tiny, first), xT(p1,h1), wrest(p1)
        nc.gpsimd.dma_start(trig4[:], trig[:])
        nc.gpsimd.dma_start(xT_sb[1][:, HS:S], xT[128:256, HS:S])
        nc.gpsimd.dma_start(w_sb[1][:, WO0:WCOLS], wbig[128:256, WO0:WCOLS])
        make_identity(nc, ident[:])

        def acc_ps():
            return pp.tile([128, 2 * D], F32, tag="acc", bufs=2, name="acc_ps")

        # broadcast the 2-unique-row trig tables to [128, 1024] via tiny
        # matmuls (fills the PE while xT h1 streams in)
        def trig_bcast():
            for ti, dst in ((0, sin_sb), (1, cos_sb)):
                base = ti * (S + 128)
                for hh in range(2):
                    ps = acc_ps()
                    nc.tensor.matmul(
                        ps[:, 0:HS],
                        trig4[:, base + S:base + S + 128],
                        trig4[:, base + hh * HS:base + (hh + 1) * HS],
                        start=True, stop=True,
                    )
                    eng = nc.scalar if hh == 0 else nc.vector
                    if eng is nc.scalar:
                        nc.scalar.activation(
                            dst[:, hh * HS:(hh + 1) * HS], ps[:, 0:HS],
                            mybir.ActivationFunctionType.Copy)
                    else:
                        nc.vector.tensor_copy(
                            dst[:, hh * HS:(hh + 1) * HS], ps[:, 0:HS])

        def wslice(base, cc, col0, ncol):
            return w_sb[cc][:, base + col0: base + col0 + ncol]

        def bias_ap(idx, cc):
            return w_sb[cc][:, BIAS0 + idx: BIAS0 + idx + 1]

        # f32 copies of the biases (tensor_scalar wants f32 scalar operands)
        biasf = [cpool.tile([128, 3], F32, tag=f"biasf{cc}", name=f"biasf{cc}")
                 for cc in range(2)]
        for cc in range(2):
            nc.gpsimd.tensor_copy(biasf[cc][:], w_sb[cc][:, 0:3])

        def bias_apf(idx, cc):
            return biasf[cc][:, idx: idx + 1]

        # fused q|v activation tiles: cols [q(1024) | v(1024)], E/O chunks
        qvE = apool.tile([128, 2 * S], BF16, tag="qvE", name="qvE")
        qvO = apool.tile([128, 2 * S], BF16, tag="qvO", name="qvO")
        rqvE = apool.tile([128, 2 * S], BF16, tag="rqvE", name="rqvE")
        rqvO = apool.tile([128, 2 * S], BF16, tag="rqvO", name="rqvO")
        qv = [qvE, qvO]
        rqv = [rqvE, rqvO]
        # rope temporaries (per half, rotated)
        t_sin = [apool.tile([128, S], BF16, tag=f"tsin{i}", name=f"tsin{i}")
                 for i in range(2)]
        t_cosE = [apool.tile([128, S], BF16, tag=f"tcE{i}", name=f"tcE{i}")
                  for i in range(2)]
        t_cosO = [apool.tile([128, S], BF16, tag=f"tcO{i}", name=f"tcO{i}")
                  for i in range(2)]
        t_sinO = [apool.tile([128, S], BF16, tag=f"tsO{i}", name=f"tsO{i}")
                  for i in range(2)]
        # natural-layout roped activations: [st 0..7] x [128 s, 256 chan]
        rq_nat = apool.tile([128, 2 * S], BF16, tag="rq_nat", name="rq_nat")
        rv_nat = apool.tile([128, 2 * S], BF16, tag="rv_nat", name="rv_nat")
        # block-diag gram (bf16) and folded weight
        bdt = [apool.tile([128, D], BF16, tag=f"bdt{ec}", name=f"bdt{ec}")
               for ec in range(2)]
        w2sb = apool.tile([128, 2 * D], BF16, tag="w2sb", name="w2sb")

        # ---- projections: qv[ac][:, p*1024 + s] = sum_d w[d, a] x[d, s] ----
        # one tensor (q or v), one half; evicts split across ACT and DVE so
        # rope (gated on the last q evict) starts as early as possible
        def project(p_idx, h, dve_evict):
            wbase = (WQ0, WV0)[p_idx]
            for ac in range(2):
                ps = proj_ps()
                for dc in range(2):
                    nc.tensor.matmul(
                        ps[:],
                        wslice(wbase, dc, ac * 128, 128),
                        xT_sb[dc][:, h * HS:(h + 1) * HS],
                        start=(dc == 0), stop=(dc == 1),
                    )
                dst = qv[ac][:, p_idx * S + h * HS: p_idx * S + (h + 1) * HS]
                if dve_evict and ac == 1:
                    nc.vector.tensor_scalar_add(dst, ps[:], bias_apf(p_idx, ac))
                else:
                    nc.scalar.activation(
                        dst, ps[:],
                        mybir.ActivationFunctionType.Identity,
                        bias=bias_ap(p_idx, ac),
                    )

        # ---- rope for tensor p_idx (0=q cols 0:S, 1=v cols S:2S) ----
        # rE = E*sin - O*cos ; rO = E*cos + O*sin.  Plain 2D [128,1024] ops
        # (3D/broadcast APs drop the DVE 2x perf mode on HW).  The O*cos
        # mult goes to Pool so DVE finishes the sub/add chain sooner.
        def rope(p_idx):
            sl = slice(p_idx * S, (p_idx + 1) * S)
            ts, tcE = t_sin[p_idx][:], t_cosE[p_idx][:]
            tcO, tsO = t_cosO[p_idx][:], t_sinO[p_idx][:]
            E = qvE[:, sl]; O = qvO[:, sl]
            rE = rqvE[:, sl]; rO = rqvO[:, sl]
            nc.vector.tensor_tensor(ts, E, sin_sb, mybir.AluOpType.mult)
            nc.vector.tensor_tensor(tcO, O, cos_sb, mybir.AluOpType.mult)
            nc.vector.tensor_tensor(rE, ts, tcO, mybir.AluOpType.subtract)
            nc.vector.tensor_tensor(tcE, E, cos_sb, mybir.AluOpType.mult)
            nc.vector.tensor_tensor(tsO, O, sin_sb, mybir.AluOpType.mult)
            nc.vector.tensor_tensor(rO, tcE, tsO, mybir.AluOpType.add)

        # ---- PE transposes into PSUM-packed natural tiles, E/O split ----
        def transp_cc(tp, h, p_idx, cc):
            for stl in range(4):
                st = h * 4 + stl
                nc.tensor.transpose(
                    tp[:, stl * D + cc * 128: stl * D + (cc + 1) * 128],
                    rqv[cc][:, p_idx * S + st * 128: p_idx * S + (st + 1) * 128],
                    ident[:],
                )

        def transp_evict(tp, h, dst_nat, evict_engine):
            if evict_engine is nc.scalar:
                nc.scalar.activation(
                    dst_nat[:, h * S:(h + 1) * S], tp[:],
                    mybir.ActivationFunctionType.Copy)
            else:
                evict_engine.tensor_copy(dst_nat[:, h * S:(h + 1) * S], tp[:])

        # ---- gram accumulation over the 4 s-tiles of half h ----
        def gram(h):
            nonlocal gram_ps
            if gram_ps is None:
                gram_ps = [acc_ps()[:, 0:D] for _ in range(2)]
            for stl in range(4):
                st = h * 4 + stl
                for ec in range(2):
                    nc.tensor.matmul(
                        gram_ps[ec],
                        rv_nat[:, st * D + ec * 128: st * D + (ec + 1) * 128],
                        rq_nat[:, st * D: (st + 1) * D],
                        start=(st == 0), stop=(st == 7),
                    )

        # ================== pipeline emission ==================
        project(0, 0, dve_evict=True)    # q h0
        trig_bcast()                     # fills PE while xT h1 streams in
        dep_filler(xT_sb[1][:, HS:HS + 128])
        dep_filler(xT_sb[0][:, HS:HS + 128])
        project(0, 1, dve_evict=True)    # q h1
        project(1, 0, dve_evict=False)   # v h0 (DVE starts rope q meanwhile)
        project(1, 1, dve_evict=False)   # v h1
        rope(0)   # q on DVE (~4.3us)
        rope(1)   # v on DVE (~4.3us more)
        # trickled fillers fire as rope ops complete, holding the clock; they
        # are ordered so no filler's dep is later than the work behind it
        dep_filler(t_sin[0][:, 0:128])
        dep_filler(t_cosE[0][:, 0:128])
        tpq = [tp_ps(), tp_ps()]
        transp_cc(tpq[0], 0, 0, 0)         # rq E tiles (after rqE ready)
        transp_cc(tpq[1], 1, 0, 0)
        dep_filler(t_sinO[0][:, 0:128])
        transp_cc(tpq[0], 0, 0, 1)         # rq O tiles
        transp_cc(tpq[1], 1, 0, 1)
        transp_evict(tpq[0], 0, rq_nat, nc.scalar)
        transp_evict(tpq[1], 1, rq_nat, nc.scalar)
        dep_filler(t_sin[1][:, 0:128])
        dep_filler(t_cosO[1][:, 0:128])
        dep_filler(t_cosE[1][:, 0:128])
        tpv = [tp_ps(), tp_ps()]
        transp_cc(tpv[0], 0, 1, 0)         # rv E tiles (after rvE ready)
        transp_cc(tpv[1], 1, 1, 0)
        dep_filler(t_sinO[1][:, 0:128])
        transp_cc(tpv[0], 0, 1, 1)         # rv O tiles
        transp_cc(tpv[1], 1, 1, 1)
        transp_evict(tpv[0], 0, rv_nat, nc.vector)
        transp_evict(tpv[1], 1, rv_nat, nc.vector)
        dep_filler(rv_nat[:, 0:128])
        dep_filler(rv_nat[:, S:S + 128])
        gram(0)
        gram(1)

        # ---- BDT = gram .* (mask/4)  (Pool) ----
        for ec in range(2):
            nc.vector.tensor_tensor(
                bdt[ec][:], gram_ps[ec], wslice(MK0, ec, 0, D),
                mybir.AluOpType.mult)

        # ---- W2[f, o] = sum_e BDT[e, f] wot[e, o] ----
        w2_ps = acc_ps()
        for fc in range(2):
            for ec in range(2):
                nc.tensor.matmul(
                    w2_ps[:, fc * D:(fc + 1) * D],
                    bdt[ec][:, fc * 128:(fc + 1) * 128],
                    wslice(WO0, ec, 0, D),
                    start=(ec == 0), stop=(ec == 1),
                )
        nc.scalar.activation(w2sb[:, 0:D], w2_ps[:, 0:D],
                             mybir.ActivationFunctionType.Copy)
        nc.vector.tensor_copy(w2sb[:, D:2 * D], w2_ps[:, D:2 * D])
        dep_filler(w2sb[:, 0:128])

        # ---- final: outT[o, s] = sum_f W2[f, o] rqT[f, s] + bo ----
        for sc in range(2):
            for oc in range(2):
                ps = fin_ps()
                for fc in range(2):
                    nc.tensor.matmul(
                        ps[:],
                        w2sb[:, fc * D + oc * 128: fc * D + (oc + 1) * 128],
                        rqv[fc][:, sc * HS:(sc + 1) * HS],
                        start=(fc == 0), stop=(fc == 1),
                    )
                ot = opool.tile([128, HS], BF16, tag="out_sb", name="out_sb")
                if (sc + oc) % 2 == 0:
                    nc.scalar.activation(
                        ot[:], ps[:],
                        mybir.ActivationFunctionType.Identity,
                        bias=bias_ap(2, oc),
                    )
                else:
                    nc.vector.tensor_scalar_add(ot[:], ps[:], bias_apf(2, oc))
                nc.sync.dma_start(
                    outT[oc * 128:(oc + 1) * 128, sc * HS:(sc + 1) * HS], ot[:])


_NC_CACHE = None


def _get_nc():
    global _NC_CACHE
    if _NC_CACHE is None:
        _NC_CACHE = build_kernel()
    return _NC_CACHE


def make_in_maps(x, wq_w, wq_b, wv_w, wv_b, wo_w, wo_b):
    trig4, mask = _host_tables()
    wq_p = np.ascontiguousarray(wq_w[PERM].T).astype(ml_dtypes.bfloat16)   # [d, a]
    wv_p = np.ascontiguousarray(wv_w[PERM].T).astype(ml_dtypes.bfloat16)
    wo_p = np.ascontiguousarray(wo_w[:, PERM].T).astype(ml_dtypes.bfloat16)  # [e, o]
    bias3 = np.stack([wq_b[PERM], wv_b[PERM], wo_b], axis=1).astype(ml_dtypes.bfloat16)
    wbig = np.ascontiguousarray(
        np.concatenate([bias3, wq_p, wv_p, wo_p, mask], axis=1))
    trig = np.ascontiguousarray(trig4)
    in_maps = []
    for b in range(B):
        in_maps.append({
            "xT": np.ascontiguousarray(x[b].T).astype(ml_dtypes.bfloat16),
            "wbig": wbig, "trig": trig,
        })
    return in_maps


TRACE = False
RUN_KWARGS = {}
LAST_RESULT = None


def kernel(x, wq_w, wq_b, wk_w, wk_b, wv_w, wv_b, wo_w, wo_b):
    global LAST_RESULT
    x = np.asarray(x, dtype=np.float32)
    in_maps = make_in_maps(x, np.asarray(wq_w, np.float32), np.asarray(wq_b, np.float32),
                           np.asarray(wv_w, np.float32), np.asarray(wv_b, np.float32),
                           np.asarray(wo_w, np.float32), np.asarray(wo_b, np.float32))
    nc = _get_nc()
    res = run_bass_kernel_spmd(nc, in_maps, core_ids=list(range(N_CORES)),
                               trace=TRACE, **RUN_KWARGS)
    LAST_RESULT = res
    outs = [np.ascontiguousarray(
        res.results[b]["outT"].astype(np.float32).T) for b in range(B)]
    return np.stack(outs).astype(np.float32)
